# revision 26
# baseline (speedup 1.0000x reference)
"""Causal self-attention (B=4, T=2048, D=1024, H=16) on 8 TRN2 NeuronCores.

Sharding: core c handles batch b=c//2 and head-group g=c%2 (8 heads).
Each core computes its heads' attention + a partial output projection
(contraction over its 512 attn channels); the host sums the two partials
per batch and adds b_out (b_qkv/b_out are zeros per the problem spec; the
qk bias is still applied via the rope STT, the v bias is dropped).

v3: bf16 matmul operands (fp32 PSUM accumulation), x/weights preloaded to
SBUF once with DMA issue spread over engines, shuffle-rope (one DVE
stream_shuffle + 2 STT + 1 add per chunk, sin sign folded into the host
table), merged two-head exp per key block, exact causal widths, softmax
normalizer from a memset ones-column in V_aug, phase-restructured
schedule: all projections first, then attention iterated qc-outer /
pair-inner with the output projection of each qc interleaved, bf16 output.

Per-core device kernel (channels-on-partitions layout):
  qk-proj   qkT[ch,T] = wqk.T @ xT         (per head-pair m-chunks)
  v-proj    V[t,ch']  = xT.T @ wv_aug       (65-wide per head: 64 v cols +
                                             a ones col set by gpsimd memset)
  rope      channel order per head [e0..15,o0..15,e16..31,o16..31];
            mms = shuffle16(psum); q' = (psum+b)*cos + (mms+b_sw)*sin_signed
  S^T       [k,q] = k'^T q' per head, 2 heads packed via tile_position
  softmax   no-max-subtraction exp (score range ~|8|), causal tri mask added
            in PSUM on diagonal 128-blocks, one [128,2,w] exp per key block
  PV        attn_aug^T[65,q] = V_aug^T @ E^T accumulated over k blocks
  norm      1/Z via DVE recip, gpsimd partition_broadcast (offset-0 dests
            only: HW ignores dest partition offsets), fused PSUM multiply
  out-proj  out[q,o] = attnT.T @ wo  (partial, bf16; host sums pair partials)
"""
import sys
import numpy as np

for _p in ("/opt/trn_rl_repo", "/root/.axon_site/_ro/trn_rl_repo"):
    if _p not in sys.path:
        sys.path.append(_p)

import ml_dtypes
import concourse.bass as bass
import concourse.bacc as bacc
import concourse.tile as tile
import concourse.mybir as mybir
from concourse import bass_utils

F32 = mybir.dt.float32
BF16 = mybir.dt.bfloat16
AF = mybir.ActivationFunctionType
ALU = mybir.AluOpType
BF = ml_dtypes.bfloat16

B, T, D, H, DK = 4, 2048, 1024, 16, 64
NC_ = 8          # cores
HPG = 8          # heads per group
NPAIR = 4        # head pairs per core
KT = 8           # 128-row k-tiles over D
XC = 512         # x/qkv t-chunk width
NXC = T // XC    # 4
QC = 512         # attention q-chunk width
NQC = T // QC    # 4
NKB = T // 128   # 16 key blocks
MASK_VAL = -30000.0
# stream_shuffle permutes within each 32-partition block (mask replicated
# across the four blocks): rotate by 16 to swap the e/o halves of a block
SWAP16_MASK = [(i + 16) % 32 for i in range(32)]

_cache = {}


def _build_nc(trace_scopes=False):
    nc = bacc.Bacc("TRN2", target_bir_lowering=False, debug=False)

    xT_d = nc.dram_tensor("xT", [D, T], BF16, kind="ExternalInput").ap()
    wqk_d = nc.dram_tensor("wqk", [D, 1024], BF16, kind="ExternalInput").ap()
    wva_d = nc.dram_tensor("wva", [D, 520], BF16, kind="ExternalInput").ap()
    wo_d = nc.dram_tensor("wo", [512, 1024], BF16, kind="ExternalInput").ap()
    bqk_d = nc.dram_tensor("bqk", [128, 8], F32, kind="ExternalInput").ap()
    bqksw_d = nc.dram_tensor("bqksw", [128, 8], F32, kind="ExternalInput").ap()
    cos_d = nc.dram_tensor("cos4", [128, T], F32, kind="ExternalInput").ap()
    sin_d = nc.dram_tensor("sin4", [128, T], BF16, kind="ExternalInput").ap()
    out_d = nc.dram_tensor("out", [T, 1024], BF16, kind="ExternalOutput").ap()

    with tile.TileContext(nc, pool_alloc_mode="queue") as tc:
        _emit(tc, nc, xT_d, wqk_d, wva_d, wo_d, bqk_d,
              bqksw_d, cos_d, sin_d, out_d)
    nc.compile()
    return nc


def _emit(tc, nc, xT_d, wqk_d, wva_d, wo_d, bqk_d,
          bqksw_d, cos_d, sin_d, out_d):
    from contextlib import ExitStack
    ctx = ExitStack()
    with ctx:
        consts = ctx.enter_context(tc.tile_pool(name="consts", bufs=1))
        vpool = ctx.enter_context(tc.tile_pool(name="vpool", bufs=1))
        qkp = ctx.enter_context(tc.tile_pool(name="qkp", bufs=16))
        t1p = ctx.enter_context(tc.tile_pool(name="t1p", bufs=3))
        ep = ctx.enter_context(tc.tile_pool(name="ep", bufs=4))
        zbp = ctx.enter_context(tc.tile_pool(name="zbp", bufs=2))
        atp = ctx.enter_context(tc.tile_pool(name="atp", bufs=16))
        wop = ctx.enter_context(tc.tile_pool(name="wop", bufs=1))
        outp = ctx.enter_context(tc.tile_pool(name="outp", bufs=3))
        ps_mm = ctx.enter_context(tc.tile_pool(name="ps_mm", bufs=2, space="PSUM"))
        ps_s = ctx.enter_context(tc.tile_pool(name="ps_s", bufs=2, space="PSUM"))
        ps_pv = ctx.enter_context(tc.tile_pool(name="ps_pv", bufs=2, space="PSUM"))

        # ------------- constants (DMA issue spread over engines) -------------
        # priority order: what phase 1 needs first
        wqk_r = wqk_d.rearrange("(k p) m -> p k m", p=128)
        wqk_ts = []
        for p in range(NPAIR):
            wt = consts.tile([128, KT, 256], BF16, tag=f"wqk{p}", name=f"wqk{p}")
            wqk_ts.append(wt)
        nc.sync.dma_start(out=wqk_ts[0][:], in_=wqk_r[:, :, 0:256])
        xT_r = xT_d.rearrange("(k p) t -> p k t", p=128)
        x_ts = []
        for tq in range(NXC):
            xt = consts.tile([128, KT, XC], BF16, tag=f"xT{tq}", name=f"xT{tq}")
            x_ts.append(xt)
        nc.scalar.dma_start(out=x_ts[0][:], in_=xT_r[:, :, 0:XC])
        cos_t = consts.tile([128, T], F32, tag="cos")
        nc.gpsimd.dma_start(out=cos_t[:], in_=cos_d)
        sin_t = consts.tile([128, T], BF16, tag="sin")
        nc.gpsimd.dma_start(out=sin_t[:], in_=sin_d)
        bqk_t = consts.tile([128, 8], F32, tag="bqk")
        nc.sync.dma_start(out=bqk_t[:], in_=bqk_d)
        bqksw_t = consts.tile([128, 8], F32, tag="bqksw")
        nc.sync.dma_start(out=bqksw_t[:], in_=bqksw_d)
        wva_t = consts.tile([128, KT, 520], BF16, tag="wva")
        nc.scalar.dma_start(out=wva_t[:], in_=wva_d.rearrange("(k p) m -> p k m", p=128))
        for tq in range(1, NXC):
            eng = (nc.sync, nc.scalar, nc.gpsimd)[tq - 1]
            eng.dma_start(out=x_ts[tq][:], in_=xT_r[:, :, tq * XC:(tq + 1) * XC])
        for p in range(1, NPAIR):
            eng = (nc.sync, nc.scalar, nc.gpsimd)[p - 1]
            eng.dma_start(out=wqk_ts[p][:], in_=wqk_r[:, :, 256 * p:256 * (p + 1)])
        wo_t = wop.tile([128, 4, 1024], BF16, tag="wo")
        nc.scalar.dma_start(out=wo_t[:], in_=wo_d.rearrange("(k p) m -> p k m", p=128))
        # additive causal tri mask [128,128]: keep where col-row>=0
        mask_t = consts.tile([128, 128], F32, tag="mask")
        nc.gpsimd.memset(mask_t[:], 0.0)
        nc.gpsimd.affine_select(
            out=mask_t[:], in_=mask_t[:], compare_op=ALU.is_ge, fill=MASK_VAL,
            base=0, pattern=[[1, 128]], channel_multiplier=-1)

        # V_aug for all 16 t-blocks: [128 tok, 16 * (8 heads * 65)]
        V_t = vpool.tile([128, NKB, 520], BF16, tag="V")

        # ---------------- phase 1: projections + rope ----------------
        qk_all = []   # [pair] -> (qp_ts, kp_ts)
        for p in range(NPAIR):
            wqk_pair = wqk_ts[p]
            qp_ts = [qkp.tile([128, QC], BF16, tag="qp", name=f"qp{p}_{i}") for i in range(NQC)]
            kp_ts = [qkp.tile([128, QC], BF16, tag="kp", name=f"kp{p}_{i}") for i in range(NQC)]
            qk_all.append((qp_ts, kp_ts))

            for tq in range(NXC):
                c0 = tq * XC
                xc = x_ts[tq]

                if p == 0:
                    # ---- v-proj for the 4 t-blocks in this chunk ----
                    for tb2 in range(XC // 128):
                        tb = tq * (XC // 128) + tb2
                        for half in range(2):
                            h0 = half * 260
                            pvm = ps_mm.tile([128, 260], F32, tag="mm")
                            for k in range(KT):
                                nc.tensor.matmul(
                                    pvm[:], lhsT=xc[:, k, tb2 * 128:(tb2 + 1) * 128],
                                    rhs=wva_t[:, k, h0:h0 + 260],
                                    start=(k == 0), stop=(k == KT - 1))
                            nc.scalar.copy(V_t[:, tb, h0:h0 + 260], pvm[:])
                        # normalizer ones-columns (wva has zero cols there)
                        nc.gpsimd.memset(V_t[:, tb, 64::65], 1.0)

                # ---- qk-proj + shuffle-rope for Q (m=0) / K (m=1) ----
                for mloc, dest in ((0, qp_ts), (1, kp_ts)):
                    msel = 2 * p + mloc
                    mmp = ps_mm.tile([128, XC], F32, tag="mm")
                    for k in range(KT):
                        nc.tensor.matmul(
                            mmp[:], lhsT=wqk_pair[:, k, mloc * 128:(mloc + 1) * 128],
                            rhs=xc[:, k, :], start=(k == 0), stop=(k == KT - 1))
                    bcol = bqk_t[:, msel:msel + 1]
                    bcol_sw = bqksw_t[:, msel:msel + 1]
                    mms = t1p.tile([128, XC], F32, tag="mms")
                    nc.vector.stream_shuffle(mms[:], mmp[:], mask=SWAP16_MASK)
                    t1 = t1p.tile([128, XC], BF16, tag="t1")
                    nc.vector.scalar_tensor_tensor(
                        t1[:], mmp[:], bcol, cos_t[:, c0:c0 + XC],
                        op0=ALU.add, op1=ALU.mult)
                    t2s = t1p.tile([128, XC], BF16, tag="t2s")
                    nc.vector.scalar_tensor_tensor(
                        t2s[:], mms[:], bcol_sw, sin_t[:, c0:c0 + XC],
                        op0=ALU.add, op1=ALU.mult)
                    nc.vector.tensor_add(dest[c0 // QC][:, 0:XC], t1[:], t2s[:])

        # ------- phase 2: attention qc-outer + interleaved out-proj -------
        at_tiles = [[None] * NQC for _ in range(NPAIR)]
        for qc in range(NQC):
            nkb = 4 * qc + 4
            for p in range(NPAIR):
                qp_ts, kp_ts = qk_all[p]
                at = atp.tile([128, QC], BF16, tag="attnT", name=f"at{p}_{qc}")
                at_tiles[p][qc] = at
                pvA = ps_pv.tile([65, QC], F32, tag="pv")
                pvB = ps_pv.tile([65, QC], F32, tag="pv")
                s_tiles = {}

                def emit_s(kb):
                    d = kb - 4 * qc
                    v0 = 0 if d < 0 else 128 * d
                    sAB = ps_s.tile([128, 2, QC], F32, tag="s")
                    kq = kp_ts[kb // 4]
                    kc0 = (kb % 4) * 128
                    qq = qp_ts[qc]
                    nc.tensor.matmul(sAB[:, 0, v0:], lhsT=kq[0:64, kc0:kc0 + 128],
                                     rhs=qq[0:64, v0:],
                                     start=True, stop=True, tile_position=(0, 0))
                    nc.tensor.matmul(sAB[:, 1, v0:], lhsT=kq[64:128, kc0:kc0 + 128],
                                     rhs=qq[64:128, v0:],
                                     start=True, stop=True, tile_position=(64, 0))
                    s_tiles[kb] = (sAB, d, v0)

                emit_s(0)
                for kb in range(nkb):
                    if kb + 1 < nkb:
                        emit_s(kb + 1)
                    sAB, d, v0 = s_tiles.pop(kb)
                    if d >= 0:
                        # causal tri mask on the diagonal 128-block (both heads)
                        mb = bass.AP(mask_t.tensor, mask_t[:].offset,
                                     [mask_t[:].ap[0], [0, 2], [1, 128]])
                        nc.vector.tensor_add(sAB[:, :, v0:v0 + 128],
                                             sAB[:, :, v0:v0 + 128], mb)
                    # one exp for both heads
                    e = ep.tile([128, 2, QC], BF16, tag="e")
                    nc.scalar.activation(e[:, :, v0:], sAB[:, :, v0:], AF.Exp,
                                         scale=0.125)
                    for hh, pv in ((0, pvA), (1, pvB)):
                        nc.tensor.matmul(pv[0:65, v0:],
                                         lhsT=V_t[:, kb, (2 * p + hh) * 65:(2 * p + hh) * 65 + 65],
                                         rhs=e[:, hh, v0:], start=(kb == 0), stop=(kb == nkb - 1))
                # normalization: at[h] = pv[0:64] * bcast(1/Z)
                zzA = zbp.tile([1, QC], F32, tag="zzA")
                nc.vector.tensor_copy(zzA[:], pvA[64:65, :])
                zzB = zbp.tile([1, QC], F32, tag="zzB")
                nc.vector.tensor_copy(zzB[:], pvB[64:65, :])
                rzA = zbp.tile([1, QC], F32, tag="rzA")
                nc.vector.reciprocal_approx_fast(rzA[:], zzA[:])
                rzB = zbp.tile([1, QC], F32, tag="rzB")
                nc.vector.reciprocal_approx_fast(rzB[:], zzB[:])
                # partition_broadcast only honors offset-0 dests on HW
                zbA = zbp.tile([64, QC], F32, tag="zbA")
                nc.gpsimd.partition_broadcast(zbA[:], rzA[:])
                zbB = zbp.tile([128, QC], F32, tag="zbB")
                nc.gpsimd.partition_broadcast(zbB[:], rzB[:])
                nc.vector.tensor_mul(at[0:64, :], pvA[0:64, :], zbA[:])
                nc.vector.tensor_copy(at[64:128, :], pvB[0:64, :])
                nc.vector.tensor_mul(at[64:128, :], at[64:128, :], zbB[64:128, :])

            # ---- out-proj for this qc's 4 query blocks ----
            for qb in range(4 * qc, 4 * qc + 4):
                if qb % 3 == 0:
                    sp = ps_s.tile([128, 2, QC], F32, tag="s")
                    poA, poB = sp[:, 0, :], sp[:, 1, :]
                elif qb % 3 == 1:
                    poAt = ps_mm.tile([128, 512], F32, tag="mm", name=f"poA{qb}")
                    poBt = ps_mm.tile([128, 512], F32, tag="mm", name=f"poB{qb}")
                    poA, poB = poAt[:], poBt[:]
                else:
                    poAt = ps_pv.tile([128, 512], F32, tag="pv", name=f"poA{qb}")
                    poBt = ps_pv.tile([128, 512], F32, tag="pv", name=f"poB{qb}")
                    poA, poB = poAt[:], poBt[:]
                for p4 in range(NPAIR):
                    lt = at_tiles[p4][qb // 4][:, (qb % 4) * 128:(qb % 4) * 128 + 128]
                    nc.tensor.matmul(poA, lhsT=lt, rhs=wo_t[:, p4, 0:512],
                                     start=(p4 == 0), stop=(p4 == NPAIR - 1))
                    nc.tensor.matmul(poB, lhsT=lt, rhs=wo_t[:, p4, 512:1024],
                                     start=(p4 == 0), stop=(p4 == NPAIR - 1))
                ot = outp.tile([128, 1024], BF16, tag="ot", name=f"ot{qb}")
                nc.vector.tensor_copy(ot[:, 0:512], poA)
                nc.vector.tensor_copy(ot[:, 512:1024], poB)
                nc.gpsimd.dma_start(out=out_d[qb * 128:(qb + 1) * 128, :], in_=ot[:])


def _prep_inputs(x, W_qkv, b_qkv, W_out, cos, sin):
    """Host-side sharding/permutation. Returns list of 8 per-core in_maps."""
    x = np.ascontiguousarray(np.asarray(x, dtype=np.float32))
    W_qkv = np.asarray(W_qkv, dtype=np.float32)
    b_qkv = np.asarray(b_qkv, dtype=np.float32)
    W_out = np.asarray(W_out, dtype=np.float32)
    cos = np.asarray(cos, dtype=np.float32)
    sin = np.asarray(sin, dtype=np.float32)

    xTs = [np.ascontiguousarray(x[b].T.astype(BF)) for b in range(B)]
    # rope tables for row layout r -> rotary index i = 16*(r%64//32) + r%16;
    # rows with (r%32)<16 hold the e-half (lo out: e*cos - o*sin), rows with
    # (r%32)>=16 hold the o-half (hi out: o*cos + e*sin). sin sign folded in.
    r = np.arange(128)
    ri = 16 * ((r % 64) // 32) + (r % 16)            # rotary pair index
    sgn = np.where((r % 32) < 16, -1.0, 1.0).astype(np.float32)
    cos4 = np.ascontiguousarray(cos.T[ri])           # [128, T]
    sin4 = np.ascontiguousarray((sin.T[ri] * sgn[:, None]).astype(BF))

    groups = []
    for g in range(2):
        heads = [g * HPG + i for i in range(HPG)]
        qk_cols = []
        for p in range(NPAIR):
            A, Bh = heads[2 * p], heads[2 * p + 1]
            for base in (0, DK):                  # q block then k block
                for h in (A, Bh):
                    for blk in range(2):          # [e0..15, o0..15] per 32-blk
                        ii = 16 * blk + np.arange(16)
                        qk_cols += list(3 * DK * h + base + 2 * ii)
                        qk_cols += list(3 * DK * h + base + 2 * ii + 1)
        qk_cols = np.array(qk_cols)
        wqk = np.ascontiguousarray(W_qkv[:, qk_cols].astype(BF))      # [1024, 1024]
        bqk = np.ascontiguousarray(b_qkv[qk_cols].reshape(8, 128).T)  # [128, 8] f32
        # bias rows in swapped order for the shuffled STT operand
        rr = np.arange(128)
        sw = (rr // 32) * 32 + (rr + 16) % 32
        bqksw = np.ascontiguousarray(bqk[sw])
        # v with zeroed normalizer cols (set to 1 on device): [1024, 8*65]
        wva = np.zeros((D, 520), np.float32)
        for i, h in enumerate(heads):
            vcols = 3 * DK * h + 2 * DK + np.arange(DK)
            wva[:, i * 65:i * 65 + 64] = W_qkv[:, vcols]
        wo = np.ascontiguousarray(W_out[g * 512:(g + 1) * 512, :].astype(BF))
        groups.append(dict(wqk=wqk, bqk=bqk, bqksw=bqksw,
                           wva=np.ascontiguousarray(wva.astype(BF)), wo=wo))

    in_maps = []
    for c in range(NC_):
        b, g = c // 2, c % 2
        gr = groups[g]
        in_maps.append({
            "xT": xTs[b], "wqk": gr["wqk"], "wva": gr["wva"],
            "wo": gr["wo"], "bqk": gr["bqk"],
            "bqksw": gr["bqksw"], "cos4": cos4, "sin4": sin4,
        })
    return in_maps


def run(x, W_qkv, b_qkv, W_out, b_out, cos, sin, trace=False, trace_cores=None):
    """Build/compile (cached), run on 8 cores, return (out, BassKernelResults)."""
    if "nc" not in _cache:
        _cache["nc"] = _build_nc()
    nc = _cache["nc"]
    in_maps = _prep_inputs(x, W_qkv, b_qkv, W_out, cos, sin)
    kw = {}
    if trace:
        kw = dict(trace=True, trace_cores=trace_cores or [0])
    res = bass_utils.run_bass_kernel_spmd(nc, in_maps, core_ids=list(range(NC_)), **kw)
    b_out = np.asarray(b_out, dtype=np.float32)
    out = np.empty((B, T, D), np.float32)
    for b in range(B):
        out[b] = (res.results[2 * b]["out"].astype(np.float32)
                  + res.results[2 * b + 1]["out"].astype(np.float32)
                  + b_out[None, :])
    return out, res


def kernel(x, W_qkv, b_qkv, W_out, b_out, cos, sin):
    out, _ = run(x, W_qkv, b_qkv, W_out, b_out, cos, sin)
    return out
